# revision 9
# baseline (speedup 1.0000x reference)
"""Discounted cumsum (y[b,h,t,d] = x[b,h,t,d] + gamma[h] * y[b,h,t-1,d]) on 8 trn2 cores.

Blocked parallel scan, pure data parallelism over the B*H=64 (b,h) pairs (8 per core).
SBUF layout per pair: [128 part = t-within-block, 32 blocks x 128 d].

Single-precision fp16 pipeline (error ~2^-11, gate is 2e-2): x cast to fp16 host-side
in scan layout (contiguous 8KB DMA lines), fp16 gamma-power coefficients, fp16 output
in scan layout; host un-permutes and upcasts. 8.4MB in + 8.4MB out per core.

Tricks:
  - Carry injection gamma^{s+1}*C[k,d] == A x (e_0 tensor gamma*C) because row 0 of
    the triangular scan matrix A is the gamma powers; the carries are ADDED INTO ROW
    0 of the X tile by the gather DMA itself (SWDGE accum_op=add), so the scan is
    ONE matmul per 4-block group with a single stationary A per pair.
  - Block sums go DIRECTLY to a [128, D] PSUM tile: matmul j writes r_j to
    partition 32*(j%4)+(j//4) using tile_position column group j%4 (4 quadrants run
    concurrently); the stationary is a 32-col sliding window of a [T, 40] constant
    whose only nonzero column is u. A zero-weight matmul clears the bank first, so
    every real matmul accumulates (start=False). No flat copies, no scatter DMA.
  - 3-stage skewed software pipeline: iteration p emits [u-pass(p), carry(p-1),
    scan(p-2)], so no engine waits on same-iteration cross-engine work and the
    carry-chain latency (GT matmul -> fp16 round -> accum-gather, ~3-4us) hides
    under two iterations. All 8 x-loads are issued up front (xin bufs=8), pair 0's
    load ahead of the GT constant; DVE/ACT alternate group evacuation with a
    per-pair phase shift; the two tail stores are split in half to shorten the
    drain.
"""

import numpy as np

B, H, S, D = 4, 16, 4096, 128
T = 128          # block length (matmul contraction dim)
KB = S // T      # 32 blocks per sequence
NG = 4           # blocks per scan matmul group (4*128 = 512 moving cols, fp32 PSUM)
G = KB // NG     # 8 groups per pair
NCORES = 8
PAIRS = (B * H) // NCORES  # 8 pair-slots per core
UW = 40          # u-window const width: 8 window positions x 32 cols

_nc_cache = {}


def _scat_row(j):
    # PSUM partition that holds block j's sum (column group j%4, column j//4)
    return 32 * (j % 4) + (j // 4)


def _build_program():
    if "nc" in _nc_cache:
        return _nc_cache["nc"]

    import concourse.bass as bass
    import concourse.mybir as mybir
    from concourse.tile import TileContext

    f32 = mybir.dt.float32
    bf16 = mybir.dt.bfloat16
    fp16 = mybir.dt.float16
    ADD = mybir.AluOpType.add

    nc = bass.Bass(trn_type="TRN2")

    x_d = nc.declare_dram_parameter("x16", [PAIRS, T, KB * D], fp16, isOutput=False)
    A_d = nc.declare_dram_parameter("A_all", [T, PAIRS * T], fp16, isOutput=False)
    U_d = nc.declare_dram_parameter("U_all", [T, PAIRS * UW], fp16, isOutput=False)
    GT_d = nc.declare_dram_parameter("GT_all", [T, PAIRS * KB], fp16, isOutput=False)
    Z_d = nc.declare_dram_parameter("Z_all", [T, T], fp16, isOutput=False)
    y_d = nc.declare_dram_parameter("y", [PAIRS, T, KB * D], fp16, isOutput=True)

    with TileContext(nc) as tc:
        with (
            tc.tile_pool(name="const", bufs=1) as cpool,
            tc.tile_pool(name="xin", bufs=8) as xpool,
            tc.tile_pool(name="yout", bufs=4) as ypool,
            tc.tile_pool(name="r32", bufs=4) as r32pool,
            tc.tile_pool(name="c32", bufs=4) as c32pool,
            tc.tile_pool(name="grp_ps", bufs=5, space="PSUM") as gp_pool,
            tc.tile_pool(name="r_ps", bufs=2, space="PSUM") as r_ps_pool,
            tc.tile_pool(name="c_ps", bufs=1, space="PSUM") as cp_pool,
        ):
            # small consts early on the SP ring (ahead of the x loads), the
            # big A matrix on the ACT ring (idle until the first store).
            uc = cpool.tile([T, PAIRS * UW], fp16, tag="uc")
            GTc = cpool.tile([T, PAIRS * KB], fp16, tag="GTc")
            Zc = cpool.tile([T, T], fp16, tag="Zc")
            Ac = cpool.tile([T, PAIRS * T], fp16, tag="Ac")
            nc.sync.dma_start(out=uc[:], in_=U_d[:])
            nc.sync.dma_start(out=Zc[:], in_=Z_d[:])
            nc.scalar.dma_start(out=Ac[:], in_=A_d[:])
            # first pair's load ahead of the bulkier GT const: u-pass(0)
            # starts ~3us earlier; remaining loads follow GTc.
            X0 = xpool.tile([T, KB * D], fp16, tag="Xh")
            nc.sync.dma_start(out=X0[:], in_=x_d[0])
            nc.sync.dma_start(out=GTc[:], in_=GT_d[:])

            def absorb(ap_src):
                # standalone bf16 ldweights: makes PE wait on that tile's DMA
                # lane here; the real matmuls self-load their own stationary.
                nc.tensor.ldweights(ap_src.bitcast(bf16))

            absorb(uc[0:1, 0:1])
            absorb(GTc[0:1, 0:1])
            absorb(Zc[0:1, 0:1])
            absorb(Ac[0:1, 0:1])

            def emit_load(p):
                if p == 0:
                    return X0
                Xh = xpool.tile([T, KB * D], fp16, tag="Xh")
                nc.sync.dma_start(out=Xh[:], in_=x_d[p])
                return Xh

            def emit_upass(p, Xh):
                # block sums straight into PSUM: r_j -> partition scat_row(j)
                R32ps = r_ps_pool.tile([T, D], f32, tag="R32ps")
                # bank-clear matmul: zero stationary, const rhs (always ready)
                nc.tensor.matmul(
                    R32ps[:], lhsT=Zc[:], rhs=Zc[:],
                    start=True, stop=False, skip_group_check=True,
                )
                ub = p * UW
                for j in range(KB):
                    q, w = j % 4, j // 4
                    nc.tensor.matmul(
                        R32ps[32 * q : 32 * q + 32, :],
                        lhsT=uc[:, ub + 8 - w : ub + UW - w],
                        rhs=Xh[:, j * D : (j + 1) * D],
                        start=False, stop=(j == KB - 1),
                        tile_position=(0, 32 * q),
                        skip_group_check=True,
                    )
                R32 = r32pool.tile([T, D], fp16, tag="R32")
                nc.vector.tensor_copy(out=R32[:], in_=R32ps[:])
                return R32

            def emit_carry(p, Xh, R32):
                # carries: gamma*C[k] = sum_j gamma*GT[j,k] r_j (GT rows are
                # host-scattered to match scat_row), then ADD into row 0 of
                # Xh during the gather (row 0 of A is the gamma powers, so
                # the scan matmul applies the injection for free).
                cp = cp_pool.tile([KB, D], f32, tag="cp")
                nc.tensor.matmul(
                    cp[:], lhsT=GTc[:, p * KB : (p + 1) * KB], rhs=R32[:],
                    start=True, stop=True,
                )
                C32h = c32pool.tile([KB, D], fp16, tag="C32h")
                nc.vector.tensor_copy(out=C32h[:], in_=cp[:])
                nc.gpsimd.dma_start(out=Xh[0:1, :], in_=C32h[:], accum_op=ADD)

            def emit_scan(p, Xh, split_store=False):
                Ys = ypool.tile([T, KB * D], fp16, tag="Ys")
                half = G // 2 * NG * D
                for g in range(G):
                    grp = gp_pool.tile([T, NG * D], f32, tag="grp")
                    sl = slice(g * NG * D, (g + 1) * NG * D)
                    nc.tensor.matmul(
                        grp[:], lhsT=Ac[:, p * T : (p + 1) * T], rhs=Xh[:, sl],
                        start=True, stop=True,
                    )
                    if (g + p) % 2 == 0:
                        nc.vector.tensor_copy(out=Ys[:, sl], in_=grp[:])
                    else:
                        nc.scalar.copy(out=Ys[:, sl], in_=grp[:])
                    if split_store and g == G // 2 - 1:
                        nc.scalar.dma_start(
                            out=y_d[p][:, 0:half], in_=Ys[:, 0:half]
                        )
                if split_store:
                    nc.scalar.dma_start(out=y_d[p][:, half:], in_=Ys[:, half:])
                else:
                    nc.scalar.dma_start(out=y_d[p], in_=Ys[:])

            pend_carry = None
            pend_scan = []
            for p in range(PAIRS):
                Xh = emit_load(p)
                R32 = emit_upass(p, Xh)
                if pend_carry is not None:
                    emit_carry(*pend_carry)
                    pend_scan.append((pend_carry[0], pend_carry[1]))
                if len(pend_scan) == 2:
                    emit_scan(*pend_scan.pop(0))
                pend_carry = (p, Xh, R32)
            emit_carry(*pend_carry)
            pend_scan.append((pend_carry[0], pend_carry[1]))
            emit_scan(*pend_scan.pop(0), split_store=True)
            emit_scan(*pend_scan.pop(0), split_store=True)

    # Split excess per-instruction sync waits onto InstEventSemaphore carriers.
    import bass_rust

    bass_rust.generate_event_semaphores(nc)

    _nc_cache["nc"] = nc
    return nc


def _host_constants(g):
    """Per-pair gamma-power constants from float64."""
    pw = np.power(g, np.arange(S, dtype=np.float64))
    t_idx = np.arange(T)
    t_minus_s = t_idx[None, :] - t_idx[:, None]
    A = np.where(t_minus_s >= 0, pw[np.clip(t_minus_s, 0, None)], 0.0)  # [s, t]
    u = pw[127 - t_idx]
    pw128 = np.power(pw[T], np.arange(KB, dtype=np.float64))
    k_minus_j = np.arange(KB)[None, :] - 1 - np.arange(KB)[:, None]
    # gamma * GT so the gathered value is exactly the row-0 injection term
    GT = g * np.where(k_minus_j >= 0, pw128[np.clip(k_minus_j, 0, None)], 0.0)
    return A, u, GT


def _make_in_maps(tensor, gamma):
    x = np.asarray(tensor, dtype=np.float32).reshape(B * H, S, D)
    gam = np.asarray(gamma, dtype=np.float64).reshape(H)

    # scan layout [s, (k, d)], one vectorized pass over all pairs
    x16 = np.ascontiguousarray(
        x.reshape(B * H, KB, T, D).transpose(0, 2, 1, 3)
    ).reshape(B * H, T, KB * D).astype(np.float16)

    in_maps = []
    for c in range(NCORES):
        A_all = np.zeros((T, PAIRS * T), np.float16)
        U_all = np.zeros((T, PAIRS * UW), np.float16)
        GT_all = np.zeros((T, PAIRS * KB), np.float16)
        for p in range(PAIRS):
            pid = c * PAIRS + p
            A, u, GT = _host_constants(gam[pid % H])
            A_all[:, p * T : (p + 1) * T] = A.astype(np.float16)
            U_all[:, p * UW + 8] = u.astype(np.float16)
            # scatter GT rows to the PSUM partition layout of the u-pass
            GTs = np.zeros((T, KB), np.float64)
            for j in range(KB):
                GTs[_scat_row(j)] = GT[j]
            GT_all[:, p * KB : (p + 1) * KB] = GTs.astype(np.float16)
        in_maps.append(
            {
                "x16": x16[c * PAIRS : (c + 1) * PAIRS],
                "A_all": A_all,
                "U_all": U_all,
                "GT_all": GT_all,
                "Z_all": np.zeros((T, T), np.float16),
            }
        )
    return in_maps


def _gather_output(results):
    ys = np.concatenate(
        [np.asarray(results[c]["y"]).reshape(PAIRS, T, KB * D) for c in range(NCORES)]
    )
    y = ys.reshape(B * H, T, KB, D).transpose(0, 2, 1, 3).astype(np.float32)
    return np.ascontiguousarray(y).reshape(B, H, S, D)


def kernel(tensor, gamma):
    from concourse.bass_utils import run_bass_kernel_spmd

    in_maps = _make_in_maps(tensor, gamma)
    nc = _build_program()
    res = run_bass_kernel_spmd(nc, in_maps, list(range(NCORES))).results
    return _gather_output(res)


# revision 10
# speedup vs baseline: 1.0300x; 1.0300x over previous
"""Discounted cumsum (y[b,h,t,d] = x[b,h,t,d] + gamma[h] * y[b,h,t-1,d]) on 8 trn2 cores.

Blocked parallel scan, pure data parallelism over the B*H=64 (b,h) pairs (8 per core).
SBUF layout per pair: [128 part = t-within-block, 32 blocks x 128 d].

Single-precision fp16 pipeline (error ~2^-11, gate is 2e-2): x cast to fp16 host-side
in scan layout (contiguous 8KB DMA lines), fp16 gamma-power coefficients, fp16 output
in scan layout; host un-permutes and upcasts. 8.4MB in + 8.4MB out per core.

Tricks:
  - Carry injection gamma^{s+1}*C[k,d] == A x (e_0 tensor gamma*C) because row 0 of
    the triangular scan matrix A is the gamma powers; the carries are ADDED INTO ROW
    0 of the X tile by the gather DMA itself (SWDGE accum_op=add), so the scan is
    ONE matmul per 4-block group with a single stationary A per pair.
  - Block sums go DIRECTLY to a [128, D] PSUM tile: matmul j writes r_j to
    partition 32*(j%4)+(j//4) using tile_position column group j%4 (4 quadrants run
    concurrently); the stationary is a 32-col sliding window of a [T, 40] constant
    whose only nonzero column is u. A zero-weight matmul clears the bank first, so
    every real matmul accumulates (start=False). No flat copies, no scatter DMA.
  - 3-stage skewed software pipeline: iteration p emits [u-pass(p), carry(p-1),
    scan(p-2)], so no engine waits on same-iteration cross-engine work and the
    carry-chain latency (GT matmul -> fp16 round -> accum-gather, ~3-4us) hides
    under two iterations. All 8 x-loads are issued up front (xin bufs=8) with
    pair 0's load ahead of the GT constant so the in-stream saturates from the
    start; the final pair's store is split in half to shorten the drain.
"""

import numpy as np

B, H, S, D = 4, 16, 4096, 128
T = 128          # block length (matmul contraction dim)
KB = S // T      # 32 blocks per sequence
NG = 4           # blocks per scan matmul group (4*128 = 512 moving cols, fp32 PSUM)
G = KB // NG     # 8 groups per pair
NCORES = 8
PAIRS = (B * H) // NCORES  # 8 pair-slots per core
UW = 40          # u-window const width: 8 window positions x 32 cols

_nc_cache = {}


def _scat_row(j):
    # PSUM partition that holds block j's sum (column group j%4, column j//4)
    return 32 * (j % 4) + (j // 4)


def _build_program():
    if "nc" in _nc_cache:
        return _nc_cache["nc"]

    import concourse.bass as bass
    import concourse.mybir as mybir
    from concourse.tile import TileContext

    f32 = mybir.dt.float32
    bf16 = mybir.dt.bfloat16
    fp16 = mybir.dt.float16
    ADD = mybir.AluOpType.add

    nc = bass.Bass(trn_type="TRN2")

    x_d = nc.declare_dram_parameter("x16", [PAIRS, T, KB * D], fp16, isOutput=False)
    A_d = nc.declare_dram_parameter("A_all", [T, PAIRS * T], fp16, isOutput=False)
    U_d = nc.declare_dram_parameter("U_all", [T, PAIRS * UW], fp16, isOutput=False)
    GT_d = nc.declare_dram_parameter("GT_all", [T, PAIRS * KB], fp16, isOutput=False)
    Z_d = nc.declare_dram_parameter("Z_all", [T, T], fp16, isOutput=False)
    y_d = nc.declare_dram_parameter("y", [PAIRS, T, KB * D], fp16, isOutput=True)

    with TileContext(nc) as tc:
        with (
            tc.tile_pool(name="const", bufs=1) as cpool,
            tc.tile_pool(name="xin", bufs=8) as xpool,
            tc.tile_pool(name="yout", bufs=3) as ypool,
            tc.tile_pool(name="r32", bufs=4) as r32pool,
            tc.tile_pool(name="c32", bufs=4) as c32pool,
            tc.tile_pool(name="grp_ps", bufs=5, space="PSUM") as gp_pool,
            tc.tile_pool(name="r_ps", bufs=2, space="PSUM") as r_ps_pool,
            tc.tile_pool(name="c_ps", bufs=1, space="PSUM") as cp_pool,
        ):
            # small consts early on the SP ring (ahead of the x loads), the
            # big A matrix on the ACT ring (idle until the first store).
            uc = cpool.tile([T, PAIRS * UW], fp16, tag="uc")
            GTc = cpool.tile([T, PAIRS * KB], fp16, tag="GTc")
            Zc = cpool.tile([T, T], fp16, tag="Zc")
            Ac = cpool.tile([T, PAIRS * T], fp16, tag="Ac")
            nc.sync.dma_start(out=uc[:], in_=U_d[:])
            nc.sync.dma_start(out=Zc[:], in_=Z_d[:])
            nc.scalar.dma_start(out=Ac[:], in_=A_d[:])
            # first pair's load ahead of the bulkier GT const: u-pass(0)
            # starts ~3us earlier; remaining loads follow GTc.
            X0 = xpool.tile([T, KB * D], fp16, tag="Xh")
            nc.sync.dma_start(out=X0[:], in_=x_d[0])
            nc.sync.dma_start(out=GTc[:], in_=GT_d[:])

            def absorb(ap_src):
                # standalone bf16 ldweights: makes PE wait on that tile's DMA
                # lane here; the real matmuls self-load their own stationary.
                nc.tensor.ldweights(ap_src.bitcast(bf16))

            absorb(uc[0:1, 0:1])
            absorb(GTc[0:1, 0:1])
            absorb(Zc[0:1, 0:1])
            absorb(Ac[0:1, 0:1])

            def emit_load(p):
                if p == 0:
                    return X0
                Xh = xpool.tile([T, KB * D], fp16, tag="Xh")
                nc.sync.dma_start(out=Xh[:], in_=x_d[p])
                return Xh

            def emit_upass(p, Xh):
                # block sums straight into PSUM: r_j -> partition scat_row(j)
                R32ps = r_ps_pool.tile([T, D], f32, tag="R32ps")
                # bank-clear matmul: zero stationary, const rhs (always ready)
                nc.tensor.matmul(
                    R32ps[:], lhsT=Zc[:], rhs=Zc[:],
                    start=True, stop=False, skip_group_check=True,
                )
                ub = p * UW
                for j in range(KB):
                    q, w = j % 4, j // 4
                    nc.tensor.matmul(
                        R32ps[32 * q : 32 * q + 32, :],
                        lhsT=uc[:, ub + 8 - w : ub + UW - w],
                        rhs=Xh[:, j * D : (j + 1) * D],
                        start=False, stop=(j == KB - 1),
                        tile_position=(0, 32 * q),
                        skip_group_check=True,
                    )
                R32 = r32pool.tile([T, D], fp16, tag="R32")
                nc.vector.tensor_copy(out=R32[:], in_=R32ps[:])
                return R32

            def emit_carry(p, Xh, R32):
                # carries: gamma*C[k] = sum_j gamma*GT[j,k] r_j (GT rows are
                # host-scattered to match scat_row), then ADD into row 0 of
                # Xh during the gather (row 0 of A is the gamma powers, so
                # the scan matmul applies the injection for free).
                cp = cp_pool.tile([KB, D], f32, tag="cp")
                nc.tensor.matmul(
                    cp[:], lhsT=GTc[:, p * KB : (p + 1) * KB], rhs=R32[:],
                    start=True, stop=True,
                )
                C32h = c32pool.tile([KB, D], fp16, tag="C32h")
                nc.vector.tensor_copy(out=C32h[:], in_=cp[:])
                nc.gpsimd.dma_start(out=Xh[0:1, :], in_=C32h[:], accum_op=ADD)

            def emit_scan(p, Xh, split_store=False):
                Ys = ypool.tile([T, KB * D], fp16, tag="Ys")
                half = G // 2 * NG * D
                for g in range(G):
                    grp = gp_pool.tile([T, NG * D], f32, tag="grp")
                    sl = slice(g * NG * D, (g + 1) * NG * D)
                    nc.tensor.matmul(
                        grp[:], lhsT=Ac[:, p * T : (p + 1) * T], rhs=Xh[:, sl],
                        start=True, stop=True,
                    )
                    if g % 2 == 0:
                        nc.vector.tensor_copy(out=Ys[:, sl], in_=grp[:])
                    else:
                        nc.scalar.copy(out=Ys[:, sl], in_=grp[:])
                    if split_store and g == G // 2 - 1:
                        nc.scalar.dma_start(
                            out=y_d[p][:, 0:half], in_=Ys[:, 0:half]
                        )
                if split_store:
                    nc.scalar.dma_start(out=y_d[p][:, half:], in_=Ys[:, half:])
                else:
                    nc.scalar.dma_start(out=y_d[p], in_=Ys[:])

            pend_carry = None
            pend_scan = []
            for p in range(PAIRS):
                Xh = emit_load(p)
                R32 = emit_upass(p, Xh)
                if pend_carry is not None:
                    emit_carry(*pend_carry)
                    pend_scan.append((pend_carry[0], pend_carry[1]))
                if len(pend_scan) == 2:
                    emit_scan(*pend_scan.pop(0))
                pend_carry = (p, Xh, R32)
            emit_carry(*pend_carry)
            pend_scan.append((pend_carry[0], pend_carry[1]))
            emit_scan(*pend_scan.pop(0))
            emit_scan(*pend_scan.pop(0), split_store=True)

    # Split excess per-instruction sync waits onto InstEventSemaphore carriers.
    import bass_rust

    bass_rust.generate_event_semaphores(nc)

    _nc_cache["nc"] = nc
    return nc


def _host_constants(g):
    """Per-pair gamma-power constants from float64."""
    pw = np.power(g, np.arange(S, dtype=np.float64))
    t_idx = np.arange(T)
    t_minus_s = t_idx[None, :] - t_idx[:, None]
    A = np.where(t_minus_s >= 0, pw[np.clip(t_minus_s, 0, None)], 0.0)  # [s, t]
    u = pw[127 - t_idx]
    pw128 = np.power(pw[T], np.arange(KB, dtype=np.float64))
    k_minus_j = np.arange(KB)[None, :] - 1 - np.arange(KB)[:, None]
    # gamma * GT so the gathered value is exactly the row-0 injection term
    GT = g * np.where(k_minus_j >= 0, pw128[np.clip(k_minus_j, 0, None)], 0.0)
    return A, u, GT


def _make_in_maps(tensor, gamma):
    x = np.asarray(tensor, dtype=np.float32).reshape(B * H, S, D)
    gam = np.asarray(gamma, dtype=np.float64).reshape(H)

    # scan layout [s, (k, d)], one vectorized pass over all pairs
    x16 = np.ascontiguousarray(
        x.reshape(B * H, KB, T, D).transpose(0, 2, 1, 3)
    ).reshape(B * H, T, KB * D).astype(np.float16)

    in_maps = []
    for c in range(NCORES):
        A_all = np.zeros((T, PAIRS * T), np.float16)
        U_all = np.zeros((T, PAIRS * UW), np.float16)
        GT_all = np.zeros((T, PAIRS * KB), np.float16)
        for p in range(PAIRS):
            pid = c * PAIRS + p
            A, u, GT = _host_constants(gam[pid % H])
            A_all[:, p * T : (p + 1) * T] = A.astype(np.float16)
            U_all[:, p * UW + 8] = u.astype(np.float16)
            # scatter GT rows to the PSUM partition layout of the u-pass
            GTs = np.zeros((T, KB), np.float64)
            for j in range(KB):
                GTs[_scat_row(j)] = GT[j]
            GT_all[:, p * KB : (p + 1) * KB] = GTs.astype(np.float16)
        in_maps.append(
            {
                "x16": x16[c * PAIRS : (c + 1) * PAIRS],
                "A_all": A_all,
                "U_all": U_all,
                "GT_all": GT_all,
                "Z_all": np.zeros((T, T), np.float16),
            }
        )
    return in_maps


def _gather_output(results):
    ys = np.concatenate(
        [np.asarray(results[c]["y"]).reshape(PAIRS, T, KB * D) for c in range(NCORES)]
    )
    y = ys.reshape(B * H, T, KB, D).transpose(0, 2, 1, 3).astype(np.float32)
    return np.ascontiguousarray(y).reshape(B, H, S, D)


def kernel(tensor, gamma):
    from concourse.bass_utils import run_bass_kernel_spmd

    in_maps = _make_in_maps(tensor, gamma)
    nc = _build_program()
    res = run_bass_kernel_spmd(nc, in_maps, list(range(NCORES))).results
    return _gather_output(res)
